# revision 10
# baseline (speedup 1.0000x reference)
"""AttentionBlock (GroupNorm + single-head self-attention over 64x64 spatial
positions + projection + residual) on 8 Trainium2 NeuronCores.

Sharding: data-parallel over the batch dim (B=4) x sequence-parallel over
query rows (2 halves of N=4096) -> 8 cores, each computing out[b][:, half].
Each core redundantly computes GroupNorm / K / V for its batch element (cheap)
and runs a flash-style attention over its 2048 query rows with all 4096 keys.

Layout notes (per core):
  xn_ext  [65, 4096]  normalized input, channels on partitions, + ones row
  q_sb    [128, 2048] q for this half, duplicated into both partition halves
  k_sb    [128, 4096] k, duplicated (enables row-packed score matmuls)
  vT_all  [128, 32, 65] v transposed per 128-key tile, + ones col (row sums)
  scores  computed transposed: s^T[m, nq] = sum_c k[c,m] q[c,n] so softmax
          reduction over keys is a matmul contraction, not a partition reduce.
  softmax has no max-subtraction: scores are ~N(0,1) by construction
  (GN output ~N(0,1), xavier-scaled weights, 1/sqrt(C) fold into Wq), so
  exp() cannot overflow fp32.
"""

import sys

import numpy as np

_REPO = "/opt/trn_rl_repo"
if _REPO not in sys.path:
    sys.path.insert(0, _REPO)

B, C, H, W = 4, 64, 64, 64
N = H * W          # 4096
NCORES = 8
NQ = N * B // NCORES   # 2048 query rows per core
NQ_CHUNK = 512
N_CHUNKS = NQ // NQ_CHUNK   # 4
MT = 128           # keys per m-tile
N_MT = N // MT     # 32
EPS = 1e-5
NUM_GROUPS = 8

_MAXW = 1  # max sync waits this walrus accepts on a TPB_CTRL (Drain)


def _install_tile_drain_fix():
    """This walrus build rejects >1 sync-wait command on a Drain (TPB_CTRL)
    instruction; Tile's exit drain carries one wait per live semaphore.
    Split the waits across a chain of drains."""
    import concourse.mybir as mybir
    import concourse.tile as tile
    from concourse.vector_clock import ScopedClock

    if getattr(tile.TileContext, "_drain_fix_installed", False):
        return

    def _patched(self, tick_clock, wait_clock):
        nc = self.nc
        drain_inst = nc.sync.drain()
        wait_clock.add_sem_waits(
            drain_inst.ins, ScopedClock({None: tick_clock.global_clock})
        )
        si = drain_inst.ins.sync_info
        if si is not None and len(si.on_wait) > _MAXW:
            waits = list(si.on_wait)
            drain_inst.ins.sync_info = mybir.SyncInfo(
                on_wait=waits[:_MAXW], on_update=list(si.on_update)
            )
            rest = waits[_MAXW:]
            for i in range(0, len(rest), _MAXW):
                d2 = nc.sync.drain()
                d2.ins.sync_info = mybir.SyncInfo(
                    on_wait=rest[i : i + _MAXW], on_update=[]
                )
        nc.all_engine_barrier()
        assert self.sems is not None
        popped = nc._tile_sem_poison_stack.pop()
        assert popped is self._sem_poison
        nc.clear_and_free_semaphores(list(self.sems.allocated().values()))
        nc.all_engine_barrier()

    tile.TileContext._drain_and_barrier = _patched
    tile.TileContext._drain_fix_installed = True


def _split_waits_json(bir_bytes, maxw=_MAXW):
    """This walrus build accepts only one sync-wait command per instruction.
    Hoist extra waits onto NoOps inserted just before the instruction (same
    engine, so the sequencer blocks identically)."""
    import json

    d = json.loads(bir_bytes)

    def walk(blk):
        out = []
        changed = False
        for ins in blk.get("instructions", []):
            si = ins.get("sync_info")
            waits = si.get("on_wait", []) if si else []
            if len(waits) > maxw:
                changed = True
                si["on_wait"] = waits[:maxw]
                rest = waits[maxw:]
                for i in range(0, len(rest), maxw):
                    out.append(
                        {
                            "engine": ins["engine"],
                            "ins": [],
                            "outs": [],
                            "name": f"{ins['name']}-sw{i}",
                            "opcode": "NoOp",
                            "sync_info": {
                                "on_update": [],
                                "on_wait": rest[i : i + maxw],
                            },
                        }
                    )
            out.append(ins)
        if changed:
            blk["instructions"] = out
        for sb in blk.get("blocks", []):
            walk(sb)

    for fn in d["functions"]:
        for blk in fn.get("blocks", []):
            walk(blk)
    return json.dumps(d).encode()


def _install_wait_split():
    import concourse.bass2jax as bass2jax
    import concourse.bass_utils as bu

    if getattr(bu, "_wait_split_installed", False):
        return
    orig = bu.compile_bir_kernel

    def wrapped(bir_json, tmpdir, neff_name="file.neff"):
        return orig(_split_waits_json(bir_json), tmpdir, neff_name=neff_name)

    bu.compile_bir_kernel = wrapped
    bass2jax.compile_bir_kernel = wrapped
    bu._wait_split_installed = True


def _build(pack=True, use_f32r=True):
    import concourse.bass as bass
    import concourse.mybir as mybir
    import concourse.tile as tile
    from concourse.bass import ts

    _install_tile_drain_fix()

    f32 = mybir.dt.float32
    f32r = mybir.dt.float32r
    AF = mybir.ActivationFunctionType
    ALU = mybir.AluOpType

    def R(ap):
        return ap.bitcast(f32r) if use_f32r else ap

    nc = bass.Bass()

    x_d = nc.dram_tensor("x", [C, N], f32, kind="ExternalInput")
    wq_d = nc.dram_tensor("wq", [C + 1, 128], f32, kind="ExternalInput")
    wk_d = nc.dram_tensor("wk", [C + 1, 128], f32, kind="ExternalInput")
    wv_d = nc.dram_tensor("wv", [C + 1, C], f32, kind="ExternalInput")
    wp_d = nc.dram_tensor("wp", [C + 1, C], f32, kind="ExternalInput")
    g_d = nc.dram_tensor("g", [C, NUM_GROUPS], f32, kind="ExternalInput")
    gt_d = nc.dram_tensor("gt", [NUM_GROUPS, C], f32, kind="ExternalInput")
    gam_d = nc.dram_tensor("gam", [C, 1], f32, kind="ExternalInput")
    bet_d = nc.dram_tensor("bet", [C, 1], f32, kind="ExternalInput")
    y_d = nc.dram_tensor("y", [C, NQ], f32, kind="ExternalOutput")

    with tile.TileContext(nc) as tc:
        with (
            tc.tile_pool(name="const", bufs=1) as const,
            tc.tile_pool(name="big", bufs=1) as big,
            tc.tile_pool(name="gn", bufs=2) as gn,
            tc.tile_pool(name="pt", bufs=3) as ptp,
            tc.tile_pool(name="sm", bufs=2) as smp,
            tc.tile_pool(name="outp", bufs=2) as outp,
        ):
            # ---------- load constants / input ----------
            wq_sb = const.tile([C + 1, 128], f32)
            wk_sb = const.tile([C + 1, 128], f32)
            wv_sb = const.tile([C + 1, C], f32)
            wp_sb = const.tile([C + 1, C], f32)
            g_sb = const.tile([C, NUM_GROUPS], f32)
            gt_sb = const.tile([NUM_GROUPS, C], f32)
            gam_sb = const.tile([C, 1], f32)
            bet_sb = const.tile([C, 1], f32)
            x_sb = big.tile([C, N], f32)
            nc.sync.dma_start(out=x_sb[:], in_=x_d[:])
            nc.sync.dma_start(out=wq_sb[:], in_=wq_d[:])
            nc.sync.dma_start(out=wk_sb[:], in_=wk_d[:])
            nc.sync.dma_start(out=wv_sb[:], in_=wv_d[:])
            nc.sync.dma_start(out=wp_sb[:], in_=wp_d[:])
            nc.sync.dma_start(out=g_sb[:], in_=g_d[:])
            nc.sync.dma_start(out=gt_sb[:], in_=gt_d[:])
            nc.sync.dma_start(out=gam_sb[:], in_=gam_d[:])
            nc.sync.dma_start(out=bet_sb[:], in_=bet_d[:])

            # ---------- GroupNorm ----------
            with tc.tile_pool(name="gnps", bufs=1, space="PSUM") as gnps:
                nsub = N // 512
                stats = gn.tile([C, nsub, 6], f32)
                xv = x_sb[:].rearrange("p (a b) -> p a b", b=512)
                for i in range(nsub):
                    nc.vector.bn_stats(out=stats[:, i, :], in_=xv[:, i, :])
                mv = gn.tile([C, 2], f32)
                nc.vector.bn_aggr(out=mv[:], in_=stats[:])
                # stats2 = [mean, E[x^2]] per channel
                stats2 = gn.tile([C, 2], f32)
                nc.vector.tensor_copy(out=stats2[:, 0:1], in_=mv[:, 0:1])
                nc.vector.tensor_mul(
                    out=stats2[:, 1:2], in0=mv[:, 0:1], in1=mv[:, 0:1]
                )
                nc.vector.tensor_add(
                    out=stats2[:, 1:2], in0=stats2[:, 1:2], in1=mv[:, 1:2]
                )
                # group aggregate then broadcast back to channels
                gs_ps = gnps.tile([NUM_GROUPS, 2], f32)
                nc.tensor.matmul(gs_ps[:], g_sb[:], stats2[:], start=True, stop=True)
                gs_sb = gn.tile([NUM_GROUPS, 2], f32)
                nc.vector.tensor_copy(out=gs_sb[:], in_=gs_ps[:])
                cs_ps = gnps.tile([C, 2], f32)
                nc.tensor.matmul(cs_ps[:], gt_sb[:], gs_sb[:], start=True, stop=True)
                # var = E[x^2] - mean^2 ; rstd = exp(-0.5*ln(var+eps))
                cs_sb = gn.tile([C, 2], f32)
                nc.vector.tensor_copy(out=cs_sb[:], in_=cs_ps[:])
                var_sb = gn.tile([C, 1], f32)
                nc.vector.tensor_mul(
                    out=var_sb[:], in0=cs_sb[:, 0:1], in1=cs_sb[:, 0:1]
                )
                nc.vector.tensor_sub(
                    out=var_sb[:], in0=cs_sb[:, 1:2], in1=var_sb[:]
                )
                rstd = gn.tile([C, 1], f32)
                eps_sb = gn.tile([C, 1], f32)
                nc.vector.memset(eps_sb[:], EPS)
                nc.scalar.activation(
                    out=rstd[:], in_=var_sb[:], func=AF.Ln, bias=eps_sb[:]
                )
                nc.scalar.activation(out=rstd[:], in_=rstd[:], func=AF.Exp, scale=-0.5)
                # A = rstd*gamma ; Bb = beta - mean*A ; xn = x*A + Bb
                a_sb = gn.tile([C, 1], f32)
                nc.vector.tensor_mul(out=a_sb[:], in0=rstd[:], in1=gam_sb[:])
                b_sb = gn.tile([C, 1], f32)
                nc.vector.tensor_mul(out=b_sb[:], in0=cs_sb[:, 0:1], in1=a_sb[:])
                nc.vector.tensor_sub(out=b_sb[:], in0=bet_sb[:], in1=b_sb[:])

                xn = big.tile([C + 1, N], f32)
                nc.vector.tensor_scalar(
                    out=xn[0:C, :],
                    in0=x_sb[:],
                    scalar1=a_sb[:],
                    scalar2=b_sb[:],
                    op0=ALU.mult,
                    op1=ALU.add,
                )
                nc.vector.memset(xn[C : C + 1, :], 1.0)

            # ---------- q / k / vT projections ----------
            q_sb = big.tile([128, NQ], f32)
            k_sb = big.tile([128, N], f32)
            vt_all = big.tile([128, N_MT, C + 1], f32)
            nc.vector.memset(vt_all[:, :, C : C + 1], 1.0)

            with tc.tile_pool(name="prjps", bufs=2, space="PSUM") as prjps:
                # q: only this core's half (host pre-rolls x so half is cols 0:NQ)
                for ci in range(N_CHUNKS):
                    sl = ts(ci, NQ_CHUNK)
                    q_ps = prjps.tile([128, NQ_CHUNK], f32)
                    nc.tensor.matmul(
                        q_ps[:], R(wq_sb[:]), R(xn[:, sl]), start=True, stop=True
                    )
                    nc.scalar.copy(out=q_sb[:, sl], in_=q_ps[:])
                # k: full sequence
                for ci in range(N // NQ_CHUNK):
                    sl = ts(ci, NQ_CHUNK)
                    k_ps = prjps.tile([128, NQ_CHUNK], f32)
                    nc.tensor.matmul(
                        k_ps[:], R(wk_sb[:]), R(xn[:, sl]), start=True, stop=True
                    )
                    nc.scalar.copy(out=k_sb[:, sl], in_=k_ps[:])
                # vT: per 128-key tile, grouped 8 tiles per PSUM bank
                for gi in range(N_MT // 8):
                    vt_ps = prjps.tile([128, 512], f32)
                    for j in range(8):
                        m = gi * 8 + j
                        nc.tensor.matmul(
                            vt_ps[:, ts(j, C)],
                            R(xn[:, ts(m, MT)]),
                            R(wv_sb[:]),
                            start=True,
                            stop=True,
                        )
                    nc.vector.tensor_copy(
                        out=vt_all[:, gi * 8 : gi * 8 + 8, 0:C],
                        in_=vt_ps[:].rearrange("p (a b) -> p a b", a=8),
                    )

            # ---------- attention main loop ----------
            o_sb = big.tile([C + 1, NQ], f32)
            nc.vector.memset(o_sb[C : C + 1, :], 1.0)

            with (
                tc.tile_pool(name="scps", bufs=2, space="PSUM") as scps,
                tc.tile_pool(name="ops", bufs=2, space="PSUM") as ops,
                tc.tile_pool(name="yps", bufs=2, space="PSUM") as yps,
                tc.tile_pool(name="sdram", bufs=2, space="DRAM") as sdram,
            ):
                for ci in range(N_CHUNKS):
                    qsl = ts(ci, NQ_CHUNK)
                    o_ps = ops.tile([C + 1, NQ_CHUNK], f32)
                    for p in range(N_MT // 2):
                        s_ps = scps.tile([128, 2 * NQ_CHUNK], f32)
                        if pack:
                            nc.tensor.matmul(
                                s_ps[:, 0:NQ_CHUNK],
                                R(k_sb[0:C, ts(2 * p, MT)]),
                                R(q_sb[0:C, qsl]),
                                start=True,
                                stop=True,
                                tile_position=(0, 0),
                            )
                            nc.tensor.matmul(
                                s_ps[:, NQ_CHUNK : 2 * NQ_CHUNK],
                                R(k_sb[C:128, ts(2 * p + 1, MT)]),
                                R(q_sb[C:128, qsl]),
                                start=True,
                                stop=True,
                                tile_position=(C, 0),
                            )
                        else:
                            for j in range(2):
                                nc.tensor.matmul(
                                    s_ps[:, ts(j, NQ_CHUNK)],
                                    R(k_sb[0:C, ts(2 * p + j, MT)]),
                                    R(q_sb[0:C, qsl]),
                                    start=True,
                                    stop=True,
                                )
                        pt = ptp.tile([128, 2 * NQ_CHUNK], f32)
                        nc.scalar.activation(out=pt[:], in_=s_ps[:], func=AF.Exp)
                        for j in range(2):
                            nc.tensor.matmul(
                                o_ps[:],
                                R(vt_all[:, 2 * p + j, :]),
                                R(pt[:, ts(j, NQ_CHUNK)]),
                                start=(p == 0 and j == 0),
                                stop=(p == N_MT // 2 - 1 and j == 1),
                            )
                    # normalize: o / S  (S = row C of o_ps, via vT ones col)
                    s_row = smp.tile([1, NQ_CHUNK], f32)
                    nc.vector.tensor_copy(out=s_row[:], in_=o_ps[C : C + 1, :])
                    nc.vector.reciprocal(out=s_row[:], in_=s_row[:])
                    # broadcast 1/S to all C partitions via a DRAM bounce
                    # (SBUF APs reject partition-step 0; DRAM APs allow it)
                    s_dr = sdram.tile([1, NQ_CHUNK], f32)
                    nc.sync.dma_start(out=s_dr[:], in_=s_row[:])
                    rs_b = smp.tile([C, NQ_CHUNK], f32)
                    bcast = bass.AP(
                        tensor=s_dr.tensor,
                        offset=s_dr.offset,
                        ap=[[0, C]] + [list(a) for a in s_dr.ap][1:],
                    )
                    nc.sync.dma_start(out=rs_b[:], in_=bcast)
                    nc.vector.tensor_mul(
                        out=o_sb[0:C, qsl], in0=o_ps[0:C, :], in1=rs_b[:]
                    )
                    # projection + residual
                    y_ps = yps.tile([C, NQ_CHUNK], f32)
                    nc.tensor.matmul(
                        y_ps[:], R(wp_sb[:]), R(o_sb[:, qsl]), start=True, stop=True
                    )
                    y_out = outp.tile([C, NQ_CHUNK], f32)
                    nc.vector.tensor_add(
                        out=y_out[:], in0=y_ps[:], in1=x_sb[:, qsl]
                    )
                    nc.sync.dma_start(out=y_d[:, qsl], in_=y_out[:])

    return nc


def _prep_inputs(x, gamma, beta, Wq, bq, Wk, bk, Wv, bv, Wp, bp):
    f = np.float32
    x = np.asarray(x, f).reshape(B, C, N)
    scale = f(1.0) / np.sqrt(f(C))
    wq = np.vstack([np.asarray(Wq, f).T, np.asarray(bq, f)[None]]) * scale
    wk = np.vstack([np.asarray(Wk, f).T, np.asarray(bk, f)[None]])
    wv = np.vstack([np.asarray(Wv, f).T, np.asarray(bv, f)[None]])
    wp = np.vstack([np.asarray(Wp, f).T, np.asarray(bp, f)[None]])
    wq2 = np.ascontiguousarray(np.hstack([wq, wq]))
    wk2 = np.ascontiguousarray(np.hstack([wk, wk]))
    # g: group-mean weights (1/#channels-per-group); gt: broadcast back
    g = np.zeros((C, NUM_GROUPS), f)
    gt = np.zeros((NUM_GROUPS, C), f)
    for c in range(C):
        g[c, c // (C // NUM_GROUPS)] = 1.0 / (C // NUM_GROUPS)
        gt[c // (C // NUM_GROUPS), c] = 1.0
    gam = np.ascontiguousarray(np.asarray(gamma, f)[:, None])
    bet = np.ascontiguousarray(np.asarray(beta, f)[:, None])

    in_maps = []
    for core in range(NCORES):
        b, half = core // 2, core % 2
        # roll x so this core's query half occupies columns [0, NQ); k/v use
        # all columns so the roll only permutes key order, which is harmless
        # for attention (softmax is order-invariant) -- but NOT harmless for
        # k-tile indexing vs. reference. It's fine: out[:, n] sums over all m.
        xb = np.roll(x[b], -half * NQ, axis=1) if half else x[b]
        in_maps.append(
            {
                "x": np.ascontiguousarray(xb),
                "wq": wq2,
                "wk": wk2,
                "wv": np.ascontiguousarray(wv),
                "wp": np.ascontiguousarray(wp),
                "g": g,
                "gt": gt,
                "gam": gam,
                "bet": bet,
            }
        )
    return in_maps


def _run(inputs, pack=True, use_f32r=True, trace=False):
    from concourse.bass_utils import run_bass_kernel_spmd

    _install_wait_split()
    nc = _build(pack=pack, use_f32r=use_f32r)
    in_maps = _prep_inputs(**inputs)
    res = run_bass_kernel_spmd(nc, in_maps, list(range(NCORES)), trace=trace)
    y = np.empty((B, C, N), np.float32)
    for core in range(NCORES):
        b, half = core // 2, core % 2
        y[b][:, half * NQ : (half + 1) * NQ] = res.results[core]["y"]
    return y.reshape(B, C, H, W), res


def kernel(**inputs):
    y, _ = _run(inputs)
    return y
